# revision 19
# baseline (speedup 1.0000x reference)
"""L2 + Chamfer distance kernel for Trainium2 (8 NeuronCores, data-parallel over batch).

Math (per reference):
  chamfer = mean_b( w_b * mean_n min_k ||adv[b,n] - ori[b,k]||^2 )
  l2      = mean_b( w_b * sqrt(sum((adv_obj[b]-ori_obj[b])^2) + EPS) )
  out     = l2 + CD_W * chamfer

The output is dominated (>99.999%) by the l2 term, so the chamfer factor
tolerates bf16 distances and a partially soft min while staying ~5e-6 rel
on the final scalar (tolerance 2e-2).

Device strategy (2 batches/core, raw bass, explicit semaphores):
  - d[n,k] = a2[n] + o2[k] - 2 a.o as ONE bf16 matmul per [128n x 512k]
    bank with a C=5 contraction: rows [-2ax,-2ay,-2az, a2, 1] x
    [ox,oy,oz, 1, o2].  Per n-tile, 8 matmuls fill all 8 PSUM banks; the
    4 row-groups (tile_position) run concurrently.
  - PSUM is drained by BOTH PSUM-capable engines, each doing a complete
    reduction independently (no cross-engine fold):
      ACT: activation(Exp, scale=-1/T, accum_out) -> softmin partial sum
           per chunk (min recovered as -T ln s on host)
      DVE: tensor_scalar(op1=min, accum_out)      -> exact chunk min
    Each n-tile's 4096 cols split [2048,1024,1024] (banks 4+2+2) -- a
    3-buffer ring so two drains + the PE fill overlap bubble-free.
    Chunk->engine assignment is greedy-balanced at build time with
    HW-measured costs (ACT FD/1.2+588ns incl its 289ns accum-read, DVE
    FD/0.96+232ns); ACT naturally takes the 2048s.
  - Per-chunk accum columns land in the output block [128, 194]:
    192 chunk cols + BPC L2 cols (sum of object diff^2 per batch, DVE).
    Host finishes: -T ln(s), min over chunks, mean over n, sqrt, weights.
  - Input DMAs fan out over 4 engine queues (gpsimd/sync/scalar/vector)
    so the first tile's operands land in ~2.5us instead of ~11us.
"""

import os
import numpy as np
import ml_dtypes

BF16 = ml_dtypes.bfloat16
B, N, K = 16, 4096, 4096
NCORES = 8
BPC = B // NCORES       # batches per core
CD_W, EPS = 0.2, 1e-7
C = 5                   # matmul contraction rows
NT = N // 128           # 32 n-tiles per batch
TILES = BPC * NT        # 64 tiles per core
CH_OFF = (0, 2048, 3072, 4096)   # chunk column offsets within a tile
CH_SZ = (2048, 1024, 1024)
CH_MM = ((0, 4), (4, 6), (6, 8))  # matmul index range per chunk
NCHUNKS = TILES * 3     # 192
SOFT_T = 0.01           # softmin temperature
OUT_COLS = NCHUNKS + BPC   # 192 chunk cols + 2 L2 cols
TILES_RUN = int(os.environ.get("KERNEL_TILES_RUN", str(TILES)))

LAST = {}               # test harness reads exec_time_ns etc. from here
_prog = None


# Per-tile chunk templates (bank counts) and engine assignment.  Even tiles
# [A:4 banks, D:2, D:2]; odd tiles [D:2, D:2, A:4] -- ACT always drains a
# 2048-col region and regions rotate tile-to-tile so the 3-buffer ring never
# stalls.  A few odd tiles flip a D-1024 to ACT to balance engine loads.
FLIP_TILES = frozenset((9, 17, 25, 33, 41, 49, 57))


def _build_schedule():
    force = os.environ.get("KERNEL_FORCE_ENGINE", "")
    tpl = []     # per tile: list of (bank_start, bank_count, engine)
    for t in range(TILES):
        if t % 2 == 0:
            ch = [(0, 4, "A"), (4, 2, "D"), (6, 2, "D")]
        else:
            e1 = "A" if t in FLIP_TILES else "D"
            ch = [(0, 2, "D"), (2, 2, e1), (4, 4, "A")]
        if force in ("A", "D"):
            ch = [(s, n, force) for (s, n, _e) in ch]
        tpl.append(ch)
    return tpl


SCHED = _build_schedule()
# flat chunk list: (tile, ci, col_off, col_len, engine)
CHUNKS = []
for _t, _ch in enumerate(SCHED):
    for _ci, (_s, _n, _e) in enumerate(_ch):
        CHUNKS.append((_t, _ci, 512 * _s, 512 * _n, _e))
ASSIGN = [c[4] for c in CHUNKS]
ENG_IDX = []
_na = _nd = 0
for _a in ASSIGN:
    if _a == "A":
        _na += 1
        ENG_IDX.append(_na)
    else:
        _nd += 1
        ENG_IDX.append(_nd)
NA_TOTAL, ND_TOTAL = _na, _nd


def _build_program():
    import concourse.bass as bass
    from concourse import mybir

    f32, bf16 = mybir.dt.float32, mybir.dt.bfloat16
    Alu = mybir.AluOpType
    Act = mybir.ActivationFunctionType

    nc = bass.Bass()
    ins = {}
    for b in range(BPC):
        ins[f"lhs{b}"] = nc.dram_tensor(f"lhs{b}", (C, N), bf16, kind="ExternalInput")
        ins[f"rhs{b}"] = nc.dram_tensor(f"rhs{b}", (C, K), bf16, kind="ExternalInput")
        ins[f"advo{b}"] = nc.dram_tensor(f"advo{b}", (128, 96), f32, kind="ExternalInput")
        ins[f"orio{b}"] = nc.dram_tensor(f"orio{b}", (128, 96), f32, kind="ExternalInput")
    out_d = nc.dram_tensor("out", (128, OUT_COLS), f32, kind="ExternalOutput")

    from contextlib import ExitStack
    with ExitStack() as _ctx:
        dma0_sem = _ctx.enter_context(nc.semaphore("dma0_sem"))   # b0 mats (gpsimd)
        dma0c_sem = _ctx.enter_context(nc.semaphore("dma0c_sem"))  # b0 mats (scalar)
        dma1_sem = _ctx.enter_context(nc.semaphore("dma1_sem"))   # batch-1 mats
        dmaf_sem = _ctx.enter_context(nc.semaphore("dmaf_sem"))   # objs + out
        pe_sem = _ctx.enter_context(nc.semaphore("pe_sem"))
        act_sem = _ctx.enter_context(nc.semaphore("act_sem"))
        dve_sem = _ctx.enter_context(nc.semaphore("dve_sem"))
        l2_sem = _ctx.enter_context(nc.semaphore("l2_sem"))
        lhs_sb = [_ctx.enter_context(nc.sbuf_tensor(f"lhs{b}_sb", [128, N], bf16))
                  for b in range(BPC)]
        rhs_sb = [_ctx.enter_context(nc.sbuf_tensor(f"rhs{b}_sb", [128, K], bf16))
                  for b in range(BPC)]
        advo_sb = [_ctx.enter_context(nc.sbuf_tensor(f"advo{b}_sb", [128, 96], f32))
                   for b in range(BPC)]
        orio_sb = [_ctx.enter_context(nc.sbuf_tensor(f"orio{b}_sb", [128, 96], f32))
                   for b in range(BPC)]
        junkA = _ctx.enter_context(nc.sbuf_tensor("junkA", [128, 2048], bf16))
        junkD = _ctx.enter_context(nc.sbuf_tensor("junkD", [128, 2048], bf16))
        diff = _ctx.enter_context(nc.sbuf_tensor("diff", [128, 96], f32))
        dsq = _ctx.enter_context(nc.sbuf_tensor("dsq", [128, 96], f32))
        out_sb = _ctx.enter_context(nc.sbuf_tensor("out_sb", [128, OUT_COLS], f32))
        pt = _ctx.enter_context(nc.psum_tensor("pt", [128, 4096], f32))

        NCH_RUN = TILES_RUN * 3
        MODE = os.environ.get("KERNEL_DEBUG_MODE", "")   # "", "nodrain", "l2only"
        DO_PE = MODE != "l2only"
        DO_DRAIN = MODE == ""
        NA_RUN = sum(1 for j in range(NCH_RUN) if ASSIGN[j] == "A") if DO_DRAIN else 0
        ND_RUN = sum(1 for j in range(NCH_RUN) if ASSIGN[j] == "D") if DO_DRAIN else 0
        # DVE does the L2 cols after this many of its own chunks (past the ramp)
        L2_AFTER = min(6, ND_RUN)

        # ---- input DMA fan-out across 4 engine queues ----
        # batch-0 mats (critical, 12): gpsimd 4, sync 4, scalar 2, vector 2
        b0 = []
        for r in range(4):
            b0.append(("lhs", 0, r, None))
            for h in range(2):
                b0.append(("rhs", 0, r, h))
        b1 = []
        for r in range(4):
            b1.append(("lhs", 1, r, None))
            for h in range(2):
                b1.append(("rhs", 1, r, h))

        def issue(eng, item, sem):
            kind, b, r, h = item
            if kind == "lhs":
                eng.dma_start(out=lhs_sb[b][32 * r:32 * r + C, :],
                              in_=ins[f"lhs{b}"][:, :]).then_inc(sem, 16)
            else:
                kc = 2048 * h + 512 * r
                eng.dma_start(out=rhs_sb[b][32 * r:32 * r + C, kc:kc + 512],
                              in_=ins[f"rhs{b}"][:, kc:kc + 512]).then_inc(sem, 16)

        with nc.Block() as block:

            NSPLIT = int(os.environ.get("KERNEL_DMA_SPLIT", "6"))

            @block.gpsimd
            def _(g):
                for item in b0[0:NSPLIT]:
                    issue(g, item, dma0_sem)
                if BPC > 1:
                    for item in b1:
                        issue(g, item, dma1_sem)
                for b in range(BPC):
                    g.dma_start(out=advo_sb[b][:, :], in_=ins[f"advo{b}"][:, :]).then_inc(dmaf_sem, 16)
                    g.dma_start(out=orio_sb[b][:, :], in_=ins[f"orio{b}"][:, :]).then_inc(dmaf_sem, 16)
                # final output once both drain streams (and L2 cols) are done
                if NA_RUN:
                    g.wait_ge(act_sem, NA_RUN)
                g.wait_ge(dve_sem, ND_RUN)
                g.wait_ge(l2_sem, BPC)
                g.dma_start(out=out_d[:, :], in_=out_sb[:, :]).then_inc(dmaf_sem, 16)
                g.wait_ge(dmaf_sem, (2 * BPC + 1) * 16)

            if DO_PE:
                @block.tensor
                def _(t):
                    t.wait_ge(dma0_sem, NSPLIT * 16)
                    if NSPLIT < 12:
                        t.wait_ge(dma0c_sem, (12 - NSPLIT) * 16)
                    for tt in range(TILES_RUN):
                        b, t_ = divmod(tt, NT)
                        if tt == NT and BPC > 1:
                            t.wait_ge(dma1_sem, 12 * 16)   # batch-1 mats
                        # per-bank: which chunk of this tile ends here (inc),
                        # and which chunks of tile tt-1 must be drained before
                        # writing it (waits, deduped as we go)
                        cur = SCHED[tt]
                        prev = SCHED[tt - 1] if tt >= 1 else None
                        bank_inc = {}
                        for ci, (s, n, _e) in enumerate(cur):
                            bank_inc[s + n - 1] = ci
                        waited = set()
                        for m in range(8):   # bank m <- k-chunk [512m, 512m+512)
                            if prev is not None and DO_DRAIN:
                                for pci, (ps, pn, pe_) in enumerate(prev):
                                    if ps <= m < ps + pn and pci not in waited:
                                        waited.add(pci)
                                        j = 3 * (tt - 1) + pci
                                        sem = act_sem if ASSIGN[j] == "A" else dve_sem
                                        t.wait_ge(sem, ENG_IDX[j])
                            r = m % 4
                            kc = 512 * m
                            mm = t.matmul(
                                out=pt[:, kc:kc + 512],
                                lhsT=lhs_sb[b][32 * r:32 * r + C, 128 * t_:128 * (t_ + 1)],
                                rhs=rhs_sb[b][32 * r:32 * r + C, kc:kc + 512],
                                start=True, stop=True,
                                tile_position=(32 * r, 0),
                            )
                            if m in bank_inc:
                                mm.then_inc(pe_sem)   # chunk (tt, ci) written

            @block.scalar
            def _(s):
                for item in b0[NSPLIT:12]:
                    issue(s, item, dma0c_sem)
                for j in range(NCH_RUN):
                    if ASSIGN[j] != "A" or not DO_DRAIN:
                        continue
                    _t, _ci, off, fd, _e = CHUNKS[j]
                    s.wait_ge(pe_sem, j + 1)
                    s.activation(out=junkA[:, 0:fd],
                                 in_=pt[:, off:off + fd],
                                 func=Act.Exp, scale=-1.0 / SOFT_T,
                                 accum_out=out_sb[:, j:j + 1]).then_inc(act_sem)

            @block.vector
            def _(v):
                v.memset(out_sb[:, :], 0.0)
                nd_done = 0
                l2_emitted = False

                def emit_l2(v):
                    v.wait_ge(dmaf_sem, 2 * BPC * 16)
                    for b in range(BPC):
                        v.tensor_tensor(out=diff[:, :], in0=advo_sb[b][:, :],
                                        in1=orio_sb[b][:, :], op=Alu.subtract)
                        v.tensor_tensor(out=dsq[:, :], in0=diff[:, :],
                                        in1=diff[:, :], op=Alu.mult)
                        v.tensor_scalar(out=dsq[:, :], in0=dsq[:, :],
                                        scalar1=1.0, scalar2=None,
                                        op0=Alu.mult, op1=Alu.add,
                                        accum_out=out_sb[:, NCHUNKS + b:NCHUNKS + b + 1]
                                        ).then_inc(l2_sem)

                for j in range(NCH_RUN):
                    if ASSIGN[j] != "D" or not DO_DRAIN:
                        continue
                    _t, _ci, off, fd, _e = CHUNKS[j]
                    v.wait_ge(pe_sem, j + 1)
                    v.tensor_scalar(out=junkD[:, 0:fd],
                                    in0=pt[:, off:off + fd],
                                    scalar1=1.0, scalar2=None,
                                    op0=Alu.mult, op1=Alu.min,
                                    accum_out=out_sb[:, j:j + 1]).then_inc(dve_sem)
                    nd_done += 1
                    if nd_done == L2_AFTER and not l2_emitted:
                        emit_l2(v)
                        l2_emitted = True
                if not l2_emitted:
                    emit_l2(v)

    return nc


def _prep_core(adv, ori, advo, orio):
    maps = {}
    for b in range(BPC):
        a = np.asarray(adv[b], np.float32)      # [N, 3]
        o = np.asarray(ori[b], np.float32)      # [K, 3]
        a2 = (a * a).sum(-1)
        o2 = (o * o).sum(-1)
        L = np.empty((C, N), BF16)
        L[0:3] = (-2.0 * a).astype(BF16).T
        L[3] = a2.astype(BF16)
        L[4] = BF16(1.0)
        R = np.empty((C, K), BF16)
        R[0:3] = o.astype(BF16).T
        R[3] = BF16(1.0)
        R[4] = o2.astype(BF16)
        maps[f"lhs{b}"] = np.ascontiguousarray(L)
        maps[f"rhs{b}"] = np.ascontiguousarray(R)
        maps[f"advo{b}"] = np.ascontiguousarray(
            np.asarray(advo[b], np.float32).reshape(128, 96))
        maps[f"orio{b}"] = np.ascontiguousarray(
            np.asarray(orio[b], np.float32).reshape(128, 96))
    return maps


def kernel(adv_pc, ori_pc, adv_obj, ori_obj, weights):
    global _prog
    from concourse.bass_utils import run_bass_kernel_spmd

    if _prog is None:
        _prog = _build_program()

    adv_pc = np.asarray(adv_pc, np.float32)
    ori_pc = np.asarray(ori_pc, np.float32)
    adv_obj = np.asarray(adv_obj, np.float32)
    ori_obj = np.asarray(ori_obj, np.float32)
    weights = np.asarray(weights, np.float32)

    in_maps = []
    for c in range(NCORES):
        s = slice(BPC * c, BPC * (c + 1))
        in_maps.append(_prep_core(adv_pc[s], ori_pc[s], adv_obj[s], ori_obj[s]))

    trace = os.environ.get("BASS_TRACE_KERNEL", "") == "1"
    r = run_bass_kernel_spmd(_prog, in_maps, core_ids=list(range(NCORES)),
                             trace=trace)
    LAST["exec_time_ns"] = r.exec_time_ns
    LAST["results"] = r

    # ---- host tail: decode chunk cols -> chamfer, L2 cols -> l2 ----
    total = 0.0
    for c in range(NCORES):
        ob = np.asarray(r.results[c]["out"], np.float64)   # [128, OUT_COLS]
        for b in range(BPC):
            gb = c * BPC + b
            mins = np.full((NT, 128), np.inf)
            for t_ in range(NT):
                tt = b * NT + t_
                for ci in range(3):
                    j = 3 * tt + ci
                    col = ob[:, j]
                    if ASSIGN[j] == "A":
                        m = -SOFT_T * np.log(np.maximum(col, 1e-35))
                    else:
                        m = col
                    mins[t_] = np.minimum(mins[t_], m)
            loss1 = mins.mean()
            l2 = np.sqrt(ob[:, NCHUNKS + b].sum() + EPS)
            total += weights[gb] * (l2 + CD_W * loss1)
    return np.array(np.float32(total / B), dtype=np.float32)


# revision 21
# speedup vs baseline: 1.1616x; 1.1616x over previous
"""L2 + Chamfer distance kernel for Trainium2 (8 NeuronCores, data-parallel over batch).

Math (per reference):
  chamfer = mean_b( w_b * mean_n min_k ||adv[b,n] - ori[b,k]||^2 )
  l2      = mean_b( w_b * sqrt(sum((adv_obj[b]-ori_obj[b])^2) + EPS) )
  out     = l2 + CD_W * chamfer

The output is dominated (>99.999%) by the l2 term, so the chamfer factor
tolerates bf16 distances and a partially soft min while staying ~5e-6 rel
on the final scalar (tolerance 2e-2).

Device strategy (2 batches/core, raw bass, explicit semaphores):
  - d[n,k] = a2[n] + o2[k] - 2 a.o as ONE bf16 matmul per [128n x 512k]
    bank with a C=5 contraction: rows [-2ax,-2ay,-2az, a2, 1] x
    [ox,oy,oz, 1, o2].  Per n-tile, 8 matmuls fill all 8 PSUM banks; the
    4 row-groups (tile_position) run concurrently.
  - PSUM is drained by BOTH PSUM-capable engines, each doing a complete
    reduction independently (no cross-engine fold):
      ACT: activation(Exp, scale=-1/T, accum_out) -> softmin partial sum
           per chunk (min recovered as -T ln s on host)
      DVE: tensor_scalar(op1=min, accum_out)      -> exact chunk min
    Each n-tile's 4096 cols split [2048,1024,1024] (banks 4+2+2) -- a
    3-buffer ring so two drains + the PE fill overlap bubble-free.
    Chunk->engine assignment is greedy-balanced at build time with
    HW-measured costs (ACT FD/1.2+588ns incl its 289ns accum-read, DVE
    FD/0.96+232ns); ACT naturally takes the 2048s.
  - Per-chunk accum columns land in the output block [128, 194]:
    192 chunk cols + BPC L2 cols (sum of object diff^2 per batch, DVE).
    Host finishes: -T ln(s), min over chunks, mean over n, sqrt, weights.
  - Input DMAs fan out over 4 engine queues (gpsimd/sync/scalar/vector)
    so the first tile's operands land in ~2.5us instead of ~11us.
"""

import os
import numpy as np
import ml_dtypes

BF16 = ml_dtypes.bfloat16
B, N, K = 16, 4096, 4096
NCORES = 8
BPC = B // NCORES       # batches per core
CD_W, EPS = 0.2, 1e-7
C = 5                   # matmul contraction rows
NT = N // 128           # 32 n-tiles per batch
TILES = BPC * NT        # 64 tiles per core
CH_OFF = (0, 2048, 3072, 4096)   # chunk column offsets within a tile
CH_SZ = (2048, 1024, 1024)
CH_MM = ((0, 4), (4, 6), (6, 8))  # matmul index range per chunk
NCHUNKS = TILES * 3     # 192
SOFT_T = 0.01           # softmin temperature
OUT_COLS = NCHUNKS + BPC   # 192 chunk cols + 2 L2 cols
TILES_RUN = int(os.environ.get("KERNEL_TILES_RUN", str(TILES)))

LAST = {}               # test harness reads exec_time_ns etc. from here
_prog = None


# Per-tile chunk layout (bank_start, bank_count, engine).  Annealed against a
# calibrated pipeline simulator: (1536,1536,1024)-col chunks with strict
# engine alternation per tile ([D,A,D] / [A,D,A]) keeps both drain engines
# saturated while every PSUM bank's serial drain->PE-refill chain stays off
# the critical path.  A few odd tiles flip the 1024 chunk to DVE for balance.
FLIP_TILES = frozenset((13, 29, 45, 61))


def _build_schedule():
    force = os.environ.get("KERNEL_FORCE_ENGINE", "")
    tpl = []     # per tile: list of (bank_start, bank_count, engine)
    for t in range(TILES):
        if t % 2 == 0:
            ch = [(0, 3, "D"), (3, 3, "A"), (6, 2, "D")]
        else:
            e2 = "D" if t in FLIP_TILES else "A"
            ch = [(0, 3, "A"), (3, 3, "D"), (6, 2, e2)]
        if force in ("A", "D"):
            ch = [(s, n, force) for (s, n, _e) in ch]
        tpl.append(ch)
    return tpl


SCHED = _build_schedule()
# flat chunk list: (tile, ci, col_off, col_len, engine)
CHUNKS = []
for _t, _ch in enumerate(SCHED):
    for _ci, (_s, _n, _e) in enumerate(_ch):
        CHUNKS.append((_t, _ci, 512 * _s, 512 * _n, _e))
ASSIGN = [c[4] for c in CHUNKS]
ENG_IDX = []
_na = _nd = 0
for _a in ASSIGN:
    if _a == "A":
        _na += 1
        ENG_IDX.append(_na)
    else:
        _nd += 1
        ENG_IDX.append(_nd)
NA_TOTAL, ND_TOTAL = _na, _nd


def _build_program():
    import concourse.bass as bass
    from concourse import mybir

    f32, bf16 = mybir.dt.float32, mybir.dt.bfloat16
    Alu = mybir.AluOpType
    Act = mybir.ActivationFunctionType

    nc = bass.Bass()
    ins = {}
    for b in range(BPC):
        ins[f"lhs{b}"] = nc.dram_tensor(f"lhs{b}", (C, N), bf16, kind="ExternalInput")
        ins[f"rhs{b}"] = nc.dram_tensor(f"rhs{b}", (C, K), bf16, kind="ExternalInput")
        ins[f"advo{b}"] = nc.dram_tensor(f"advo{b}", (128, 96), f32, kind="ExternalInput")
        ins[f"orio{b}"] = nc.dram_tensor(f"orio{b}", (128, 96), f32, kind="ExternalInput")
    out_d = nc.dram_tensor("out", (128, OUT_COLS), f32, kind="ExternalOutput")

    from contextlib import ExitStack
    with ExitStack() as _ctx:
        dma0_sem = _ctx.enter_context(nc.semaphore("dma0_sem"))   # b0 mats (gpsimd)
        dma0c_sem = _ctx.enter_context(nc.semaphore("dma0c_sem"))  # b0 mats (scalar)
        dma1_sem = _ctx.enter_context(nc.semaphore("dma1_sem"))   # batch-1 mats
        dmaf_sem = _ctx.enter_context(nc.semaphore("dmaf_sem"))   # objs + out
        pe_sem = _ctx.enter_context(nc.semaphore("pe_sem"))
        act_sem = _ctx.enter_context(nc.semaphore("act_sem"))
        dve_sem = _ctx.enter_context(nc.semaphore("dve_sem"))
        l2_sem = _ctx.enter_context(nc.semaphore("l2_sem"))
        lhs_sb = [_ctx.enter_context(nc.sbuf_tensor(f"lhs{b}_sb", [128, N], bf16))
                  for b in range(BPC)]
        rhs_sb = [_ctx.enter_context(nc.sbuf_tensor(f"rhs{b}_sb", [128, K], bf16))
                  for b in range(BPC)]
        advo_sb = [_ctx.enter_context(nc.sbuf_tensor(f"advo{b}_sb", [128, 96], f32))
                   for b in range(BPC)]
        orio_sb = [_ctx.enter_context(nc.sbuf_tensor(f"orio{b}_sb", [128, 96], f32))
                   for b in range(BPC)]
        junkA = _ctx.enter_context(nc.sbuf_tensor("junkA", [128, 2048], bf16))
        junkD = _ctx.enter_context(nc.sbuf_tensor("junkD", [128, 2048], bf16))
        diff = _ctx.enter_context(nc.sbuf_tensor("diff", [128, 96], f32))
        dsq = _ctx.enter_context(nc.sbuf_tensor("dsq", [128, 96], f32))
        out_sb = _ctx.enter_context(nc.sbuf_tensor("out_sb", [128, OUT_COLS], f32))
        pt = _ctx.enter_context(nc.psum_tensor("pt", [128, 4096], f32))

        NCH_RUN = TILES_RUN * 3
        MODE = os.environ.get("KERNEL_DEBUG_MODE", "")   # "", "nodrain", "l2only"
        DO_PE = MODE != "l2only"
        DO_DRAIN = MODE == ""
        NA_RUN = sum(1 for j in range(NCH_RUN) if ASSIGN[j] == "A") if DO_DRAIN else 0
        ND_RUN = sum(1 for j in range(NCH_RUN) if ASSIGN[j] == "D") if DO_DRAIN else 0
        # DVE does the L2 cols after this many of its own chunks (past the ramp)
        L2_AFTER = min(8, ND_RUN)

        # ---- input DMA fan-out: two queues (gpsimd: even row-groups,
        # scalar: odd row-groups), ordered so tile 0 can start after the
        # first two items of each queue ----
        b0g = [("lhs", 0, 0, None), ("rhs", 0, 0, 0), ("lhs", 0, 2, None),
               ("rhs", 0, 2, 0), ("rhs", 0, 0, 1), ("rhs", 0, 2, 1)]
        b0c = [("lhs", 0, 1, None), ("rhs", 0, 1, 0), ("lhs", 0, 3, None),
               ("rhs", 0, 3, 0), ("rhs", 0, 1, 1), ("rhs", 0, 3, 1)]
        b1 = []
        for r in range(4):
            b1.append(("lhs", 1, r, None))
            for h in range(2):
                b1.append(("rhs", 1, r, h))

        def issue(eng, item, sem):
            kind, b, r, h = item
            if kind == "lhs":
                eng.dma_start(out=lhs_sb[b][32 * r:32 * r + C, :],
                              in_=ins[f"lhs{b}"][:, :]).then_inc(sem, 16)
            else:
                kc = 2048 * h + 512 * r
                eng.dma_start(out=rhs_sb[b][32 * r:32 * r + C, kc:kc + 512],
                              in_=ins[f"rhs{b}"][:, kc:kc + 512]).then_inc(sem, 16)

        with nc.Block() as block:

            @block.gpsimd
            def _(g):
                for item in b0g:
                    issue(g, item, dma0_sem)
                if BPC > 1:
                    for item in b1:
                        issue(g, item, dma1_sem)
                for b in range(BPC):
                    g.dma_start(out=advo_sb[b][:, :], in_=ins[f"advo{b}"][:, :]).then_inc(dmaf_sem, 16)
                    g.dma_start(out=orio_sb[b][:, :], in_=ins[f"orio{b}"][:, :]).then_inc(dmaf_sem, 16)
                # final output once both drain streams (and L2 cols) are done
                if NA_RUN:
                    g.wait_ge(act_sem, NA_RUN)
                g.wait_ge(dve_sem, ND_RUN)
                g.wait_ge(l2_sem, BPC)
                g.dma_start(out=out_d[:, :], in_=out_sb[:, :]).then_inc(dmaf_sem, 16)
                g.wait_ge(dmaf_sem, (2 * BPC + 1) * 16)

            if DO_PE:
                @block.tensor
                def _(t):
                    for tt in range(TILES_RUN):
                        b, t_ = divmod(tt, NT)
                        if tt == NT and BPC > 1:
                            t.wait_ge(dma1_sem, 12 * 16)   # batch-1 mats
                        # per-bank: which chunk of this tile ends here (inc),
                        # and which chunks of tile tt-1 must be drained before
                        # writing it (waits, deduped as we go)
                        cur = SCHED[tt]
                        prev = SCHED[tt - 1] if tt >= 1 else None
                        bank_inc = {}
                        for ci, (s, n, _e) in enumerate(cur):
                            bank_inc[s + n - 1] = ci
                        if tt == 0:
                            # DMA completions within a queue can land out of
                            # order (parallel SDMA engines) -- wait for full
                            # per-queue counts only
                            t.wait_ge(dma0_sem, 6 * 16)
                            t.wait_ge(dma0c_sem, 6 * 16)
                        waited = set()
                        for m in range(8):   # bank m <- k-chunk [512m, 512m+512)
                            if prev is not None and DO_DRAIN:
                                for pci, (ps, pn, pe_) in enumerate(prev):
                                    if ps <= m < ps + pn and pci not in waited:
                                        waited.add(pci)
                                        j = 3 * (tt - 1) + pci
                                        sem = act_sem if ASSIGN[j] == "A" else dve_sem
                                        t.wait_ge(sem, ENG_IDX[j])
                            r = m % 4
                            kc = 512 * m
                            mm = t.matmul(
                                out=pt[:, kc:kc + 512],
                                lhsT=lhs_sb[b][32 * r:32 * r + C, 128 * t_:128 * (t_ + 1)],
                                rhs=rhs_sb[b][32 * r:32 * r + C, kc:kc + 512],
                                start=True, stop=True,
                                tile_position=(32 * r, 0),
                            )
                            if m in bank_inc:
                                mm.then_inc(pe_sem)   # chunk (tt, ci) written

            @block.scalar
            def _(s):
                for item in b0c:
                    issue(s, item, dma0c_sem)
                for j in range(NCH_RUN):
                    if ASSIGN[j] != "A" or not DO_DRAIN:
                        continue
                    _t, _ci, off, fd, _e = CHUNKS[j]
                    s.wait_ge(pe_sem, j + 1)
                    s.activation(out=junkA[:, 0:fd],
                                 in_=pt[:, off:off + fd],
                                 func=Act.Exp, scale=-1.0 / SOFT_T,
                                 accum_out=out_sb[:, j:j + 1]).then_inc(act_sem)

            @block.vector
            def _(v):
                v.memset(out_sb[:, :], 0.0)
                nd_done = 0
                l2_emitted = False

                def emit_l2(v):
                    v.wait_ge(dmaf_sem, 2 * BPC * 16)
                    for b in range(BPC):
                        v.tensor_tensor(out=diff[:, :], in0=advo_sb[b][:, :],
                                        in1=orio_sb[b][:, :], op=Alu.subtract)
                        v.tensor_tensor(out=dsq[:, :], in0=diff[:, :],
                                        in1=diff[:, :], op=Alu.mult)
                        v.tensor_scalar(out=dsq[:, :], in0=dsq[:, :],
                                        scalar1=1.0, scalar2=None,
                                        op0=Alu.mult, op1=Alu.add,
                                        accum_out=out_sb[:, NCHUNKS + b:NCHUNKS + b + 1]
                                        ).then_inc(l2_sem)

                use_tr = os.environ.get("KERNEL_DVE_TR", "1") == "1"
                X = mybir.AxisListType.X
                for j in range(NCH_RUN):
                    if ASSIGN[j] != "D" or not DO_DRAIN:
                        continue
                    _t, _ci, off, fd, _e = CHUNKS[j]
                    v.wait_ge(pe_sem, j + 1)
                    if use_tr:
                        v.tensor_reduce(out=out_sb[:, j:j + 1],
                                        in_=pt[:, off:off + fd],
                                        axis=X, op=Alu.min).then_inc(dve_sem)
                    else:
                        v.tensor_scalar(out=junkD[:, 0:fd],
                                        in0=pt[:, off:off + fd],
                                        scalar1=1.0, scalar2=None,
                                        op0=Alu.mult, op1=Alu.min,
                                        accum_out=out_sb[:, j:j + 1]).then_inc(dve_sem)
                    nd_done += 1
                    if nd_done == L2_AFTER and not l2_emitted:
                        emit_l2(v)
                        l2_emitted = True
                if not l2_emitted:
                    emit_l2(v)

    return nc


def _prep_core(adv, ori, advo, orio):
    maps = {}
    for b in range(BPC):
        a = np.asarray(adv[b], np.float32)      # [N, 3]
        o = np.asarray(ori[b], np.float32)      # [K, 3]
        a2 = (a * a).sum(-1)
        o2 = (o * o).sum(-1)
        L = np.empty((C, N), BF16)
        L[0:3] = (-2.0 * a).astype(BF16).T
        L[3] = a2.astype(BF16)
        L[4] = BF16(1.0)
        R = np.empty((C, K), BF16)
        R[0:3] = o.astype(BF16).T
        R[3] = BF16(1.0)
        R[4] = o2.astype(BF16)
        maps[f"lhs{b}"] = np.ascontiguousarray(L)
        maps[f"rhs{b}"] = np.ascontiguousarray(R)
        maps[f"advo{b}"] = np.ascontiguousarray(
            np.asarray(advo[b], np.float32).reshape(128, 96))
        maps[f"orio{b}"] = np.ascontiguousarray(
            np.asarray(orio[b], np.float32).reshape(128, 96))
    return maps


def kernel(adv_pc, ori_pc, adv_obj, ori_obj, weights):
    global _prog
    from concourse.bass_utils import run_bass_kernel_spmd

    if _prog is None:
        _prog = _build_program()

    adv_pc = np.asarray(adv_pc, np.float32)
    ori_pc = np.asarray(ori_pc, np.float32)
    adv_obj = np.asarray(adv_obj, np.float32)
    ori_obj = np.asarray(ori_obj, np.float32)
    weights = np.asarray(weights, np.float32)

    in_maps = []
    for c in range(NCORES):
        s = slice(BPC * c, BPC * (c + 1))
        in_maps.append(_prep_core(adv_pc[s], ori_pc[s], adv_obj[s], ori_obj[s]))

    trace = os.environ.get("BASS_TRACE_KERNEL", "") == "1"
    r = run_bass_kernel_spmd(_prog, in_maps, core_ids=list(range(NCORES)),
                             trace=trace)
    LAST["exec_time_ns"] = r.exec_time_ns
    LAST["results"] = r

    # ---- host tail: decode chunk cols -> chamfer, L2 cols -> l2 ----
    total = 0.0
    for c in range(NCORES):
        ob = np.asarray(r.results[c]["out"], np.float64)   # [128, OUT_COLS]
        for b in range(BPC):
            gb = c * BPC + b
            mins = np.full((NT, 128), np.inf)
            for t_ in range(NT):
                tt = b * NT + t_
                for ci in range(3):
                    j = 3 * tt + ci
                    col = ob[:, j]
                    if ASSIGN[j] == "A":
                        m = -SOFT_T * np.log(np.maximum(col, 1e-35))
                    else:
                        m = col
                    mins[t_] = np.minimum(mins[t_], m)
            loss1 = mins.mean()
            l2 = np.sqrt(ob[:, NCHUNKS + b].sum() + EPS)
            total += weights[gb] * (l2 + CD_W * loss1)
    return np.array(np.float32(total / B), dtype=np.float32)
